# revision 1
# baseline (speedup 1.0000x reference)
"""DGCNN object encoder on 8 Trainium2 NeuronCores (Bass/Tile).

Data-parallel over batch: 16 samples -> 2 per core, SPMD program.

Per sample, each EdgeConv block is reformulated to avoid materializing
[2C, N, k] edge features:
    y[o,n] = max_{j in knn(n)} LReLU( scale_o * (Wa (x_j - x_n) + Wb x_n)_o + shift_o )
           = LReLU( max_j Utilde[o,j]  +  Vtilde[o,n] )
  with Utilde = (diag(scale) Wa) X           [O, N]
       Vtilde = (diag(scale)(Wb - Wa)) X + shift
  (LReLU is monotone; the max over neighbors only touches Utilde[o, j].)

kNN selection per 128-row tile:
  scores s[n,m] = 2 x_n.x_m - |x_m|^2  (the -|x_n|^2 term is constant per row
  and cannot change the row-wise top-k). The -|x_m|^2 term rides along as an
  augmented contraction row: lhsT = [2X; ones], rhs = [X; -sq] -> PE computes
  the full score matrix into PSUM; ScalarE copies it to SBUF.  Top-20 of each
  row via 3 rounds of DVE max8/max_index/match_replace; neighbor max of
  Utilde columns via one GPSIMD ap_gather per o-tile over two 16-wide index
  windows (ranks 1-16 and ranks 5-20; duplicates are harmless under max).
"""
import os
import sys
import time

sys.path.insert(0, "/opt/trn_rl_repo")

import numpy as np
import concourse.bass as bass
import concourse.bacc as bacc
import concourse.tile as tile
from concourse import mybir
from concourse import bass_utils

F32 = mybir.dt.float32
I16 = mybir.dt.int16
U32 = mybir.dt.uint32
AF = mybir.ActivationFunctionType
OP = mybir.AluOpType
AX = mybir.AxisListType

N = 2048
K = 20
B = 16
NCORES = 8
SPC = B // NCORES          # samples per core
EPS = 1e-5
NEG = -3.0e38
NT = N // 128              # n-tiles per sample

# (O, C_in) for edge blocks 1..4; block5: 512 -> 256
EDGE_DIMS = [(64, 3), (64, 64), (128, 64), (256, 128)]
O5, C5 = 256, 512

LAST_EXEC_NS = None
LAST_RESULTS = None


def _cdiv(a, b):
    return (a + b - 1) // b


def _edge_block(nc, tc, pools, bi, x_sb, C, O, wa_d, wb_d, sh_d, ident_sb,
                ones_row, dbg=None):
    """Emit one EdgeConv block.

    x_sb: sbuf tile holding the block input in rows [0:C].  For C < 128 the
    tile has C+1 rows and this function writes -|x_m|^2 into row C (augmented
    Gram).  For C == 128 the -sq row lives in a separate [1, N] tile and the
    Gram matmul accumulates a K=1 product.
    Returns list of o-tiles holding the block output in rows [0:128].
    """
    sb, ps, psT, dr, pers = pools
    not_ = _cdiv(O, 128)
    aug = C < 128

    # --- weights ---
    wa_sb = sb.tile([C, O], F32, tag="wa")
    nc.sync.dma_start(out=wa_sb, in_=wa_d)
    wb_sb = sb.tile([C, O], F32, tag="wb")
    nc.sync.dma_start(out=wb_sb, in_=wb_d)
    sh_sb = []
    for ot in range(not_):
        o0, o1 = ot * 128, min(O, ot * 128 + 128)
        t = sb.tile([o1 - o0, 1], F32, tag=f"sh{ot}")
        nc.sync.dma_start(out=t, in_=sh_d[o0:o1, :])
        sh_sb.append(t)

    x = x_sb[0:C, :]

    # --- A: squared norms -> -sq row ---
    xsq = sb.tile([C, N], F32, tag="work2048")
    nc.vector.tensor_mul(xsq, x, x)
    ones_sb = sb.tile([C, 1], F32, tag="ones")
    nc.vector.memset(ones_sb, 1.0)
    sq_ps = ps.tile([1, N], F32, tag="mm")
    for c in range(4):
        nc.tensor.matmul(sq_ps[:, c * 512:(c + 1) * 512], ones_sb,
                         xsq[:, c * 512:(c + 1) * 512], start=True, stop=True)
    negsq = pers.tile([1, N], F32, tag="negsq")
    nc.scalar.activation(out=negsq, in_=sq_ps, func=AF.Copy, scale=-1.0)
    if aug:
        # engine writes must start at a 32-aligned partition; DMA can place
        # the augmented row at partition C directly
        nc.sync.dma_start(out=x_sb[C:C + 1, :], in_=negsq)

    # --- A: lhsT for Gram: [2X; ones] ---
    kk = C + 1 if aug else C
    x2s = pers.tile([kk, N], F32, tag="x2s")
    nc.scalar.activation(out=x2s[0:C, :], in_=x, func=AF.Copy, scale=2.0)
    if aug:
        nc.sync.dma_start(out=x2s[C:C + 1, :], in_=ones_row)

    # --- A: U, V per o-tile ---
    u_sb, v_sb, m_sb = [], [], []
    for ot in range(not_):
        o0, o1 = ot * 128, min(O, ot * 128 + 128)
        up = ps.tile([o1 - o0, N], F32, tag="mm")
        for c in range(4):
            nc.tensor.matmul(up[:, c * 512:(c + 1) * 512], wa_sb[:, o0:o1],
                             x[:, c * 512:(c + 1) * 512], start=True, stop=True)
        u = pers.tile([o1 - o0, N], F32, tag=f"u{ot}")
        nc.scalar.activation(out=u, in_=up, func=AF.Copy, scale=1.0)
        u_sb.append(u)

        vp = ps.tile([o1 - o0, N], F32, tag="mm")
        for c in range(4):
            nc.tensor.matmul(vp[:, c * 512:(c + 1) * 512], wb_sb[:, o0:o1],
                             x[:, c * 512:(c + 1) * 512], start=True, stop=True)
        v = pers.tile([o1 - o0, N], F32, tag=f"v{ot}")
        nc.scalar.activation(out=v, in_=vp, func=AF.Identity, bias=sh_sb[ot], scale=1.0)
        v_sb.append(v)
        # block output rows [0:128]; +1 aug row when feeding a C<128 block
        rows = (o1 - o0) + (1 if (bi in (1, 2) and ot == 0) else 0)
        m = pers.tile([rows, N], F32, tag=f"b{bi}m{ot}")
        m_sb.append(m)

    # --- B: per n-tile ---
    for t in range(NT):
        n0 = t * 128
        pd_ps = ps.tile([128, N], F32, tag="mm")
        for c in range(4):
            cs = slice(c * 512, (c + 1) * 512)
            if aug:
                nc.tensor.matmul(pd_ps[:, cs], x2s[:, n0:n0 + 128],
                                 x_sb[0:C + 1, cs], start=True, stop=True)
            else:
                nc.tensor.matmul(pd_ps[:, cs], x2s[:, n0:n0 + 128],
                                 x[:, cs], start=True, stop=False)
                nc.tensor.matmul(pd_ps[:, cs], ones_row[:, n0:n0 + 128],
                                 negsq[:, cs], start=False, stop=True)
        pd_sb = sb.tile([128, N], F32, tag="work2048")
        nc.scalar.activation(out=pd_sb, in_=pd_ps, func=AF.Copy, scale=1.0)
        # top-24 (need 20) in 3 rounds, in place
        idx24 = sb.tile([128, 24], U32, tag="idx24")
        m8 = sb.tile([128, 8], F32, tag="m8")
        for r in range(3):
            nc.vector.max(out=m8, in_=pd_sb)
            nc.vector.max_index(out=idx24[:, r * 8:(r + 1) * 8], in_max=m8, in_values=pd_sb)
            if r < 2:
                nc.vector.match_replace(out=pd_sb, in_to_replace=m8, in_values=pd_sb,
                                        imm_value=NEG)
        if dbg is not None:
            nc.sync.dma_start(out=dbg[t * 128:(t + 1) * 128, :], in_=idx24)
        # windows A = ranks 1-16, B = ranks 5-20
        idxf = sb.tile([128, 32], F32, tag="idxf")
        nc.vector.tensor_copy(idxf[:, 0:16], idx24[:, 0:16])
        nc.vector.tensor_copy(idxf[:, 16:32], idx24[:, 4:20])
        idxT_ps = psT.tile([32, 128], F32, tag="idxT")
        nc.tensor.transpose(idxT_ps, idxf, ident_sb)
        idxT = sb.tile([32, 128], I16, tag="idxT")
        nc.vector.tensor_copy(idxT, idxT_ps)
        idxT_dr = dr.tile([32, 128], I16, tag="idxT_dr")
        nc.sync.dma_start(out=idxT_dr, in_=idxT)
        gidx = sb.tile([128, 256], I16, tag="gidx")
        for fo, base in ((0, 0), (128, 16 * 128)):
            rd = bass.AP(tensor=idxT_dr.tensor, offset=idxT_dr.offset + base,
                         ap=[[0, 8], [128, 16], [1, 128]])
            nc.sync.dma_start(out=gidx[:, fo:fo + 128], in_=rd)
        for ot in range(not_):
            oc = min(O, 128)
            gu = sb.tile([oc, 2 * N], F32, tag="gu")
            nc.gpsimd.ap_gather(out_ap=gu, in_ap=u_sb[ot][0:oc, :], idxs_ap=gidx[0:oc, :],
                                channels=oc, num_elems=N, d=1, num_idxs=2 * N)
            red = sb.tile([oc, 256], F32, tag="gred")
            nc.vector.tensor_reduce(out=red,
                                    in_=gu.rearrange("o (h n k) -> o h n k", h=2, k=16),
                                    axis=AX.X, op=OP.max)
            nc.vector.tensor_tensor(out=m_sb[ot][0:oc, n0:n0 + 128],
                                    in0=red[:, 0:128], in1=red[:, 128:256], op=OP.max)

    # --- C: out = lrelu(M + V), in place into M tiles ---
    for ot in range(not_):
        oc = min(O, 128)
        mm = m_sb[ot][0:oc, :]
        nc.vector.tensor_tensor(out=mm, in0=mm, in1=v_sb[ot], op=OP.add)
        nc.vector.scalar_tensor_tensor(out=mm, in0=mm, scalar=0.2,
                                       in1=mm, op0=OP.mult, op1=OP.max)
    return m_sb


def build_program(num_devices=NCORES, debug_idx=False, repeat=1):
    nc = bacc.Bacc("TRN2", target_bir_lowering=False, debug=False,
                   num_devices=num_devices)
    x_d = nc.dram_tensor("x", [SPC, 3, N], F32, kind="ExternalInput").ap()
    ident_d = nc.dram_tensor("ident", [128, 128], F32, kind="ExternalInput").ap()
    ones_d = nc.dram_tensor("ones_row", [1, N], F32, kind="ExternalInput").ap()
    w_d = {}
    for i, (O, C) in enumerate(EDGE_DIMS, start=1):
        w_d[f"wa{i}"] = nc.dram_tensor(f"wa{i}", [C, O], F32, kind="ExternalInput").ap()
        w_d[f"wb{i}"] = nc.dram_tensor(f"wb{i}", [C, O], F32, kind="ExternalInput").ap()
        w_d[f"sh{i}"] = nc.dram_tensor(f"sh{i}", [O, 1], F32, kind="ExternalInput").ap()
    w_d["w5"] = nc.dram_tensor("w5", [C5, O5], F32, kind="ExternalInput").ap()
    w_d["sh5"] = nc.dram_tensor("sh5", [O5, 1], F32, kind="ExternalInput").ap()
    out_d = nc.dram_tensor("out", [SPC, O5], F32, kind="ExternalOutput").ap()
    dbg_d = None
    if debug_idx:
        dbg_d = {}
        for s in range(SPC):
            for bi in range(1, 5):
                dbg_d[(s, bi)] = nc.dram_tensor(
                    f"dbg_idx_s{s}_b{bi}", [N, 24], U32, kind="ExternalOutput").ap()

    with tile.TileContext(nc) as tc:
        with tc.tile_pool(name="sb", bufs=2) as sb, \
             tc.tile_pool(name="ps", bufs=1, space="PSUM") as ps, \
             tc.tile_pool(name="psT", bufs=2, space="PSUM") as psT, \
             tc.tile_pool(name="dr", bufs=2, space="DRAM") as dr, \
             tc.tile_pool(name="pers", bufs=1) as pers, \
             tc.tile_pool(name="cst", bufs=1) as cst:
            pools = (sb, ps, psT, dr, pers)
            ident_sb = cst.tile([128, 128], F32)
            nc.sync.dma_start(out=ident_sb, in_=ident_d)
            ones_row = cst.tile([1, N], F32)
            nc.sync.dma_start(out=ones_row, in_=ones_d)

            for s in [i % SPC for i in range(SPC * repeat)]:
                x0 = pers.tile([4, N], F32, tag="x0")
                nc.sync.dma_start(out=x0[0:3, :], in_=x_d[s, :, :])
                xs = [x0]
                for bi, (O, C) in enumerate(EDGE_DIMS, start=1):
                    x_in = xs[-1]
                    assert not isinstance(x_in, list)
                    out_tiles = _edge_block(
                        nc, tc, pools, bi, x_in, C, O,
                        w_d[f"wa{bi}"], w_d[f"wb{bi}"], w_d[f"sh{bi}"], ident_sb,
                        ones_row,
                        dbg=None if dbg_d is None else dbg_d[(s, bi)])
                    xs.append(out_tiles if len(out_tiles) > 1 else out_tiles[0])

                # --- block 5: y = W5 @ cat(x1..x4); lrelu after global max ---
                x1, x2, x3 = xs[1], xs[2], xs[3]
                x4a, x4b = xs[4][0], xs[4][1]
                # load w5 as five part-aligned k-tiles matching the x parts
                krows = [(x1, 0, 64), (x2, 64, 128), (x3, 128, 256),
                         (x4a, 256, 384), (x4b, 384, 512)]
                w5_sb = []
                for pi, (xp, k0, k1) in enumerate(krows):
                    t = sb.tile([k1 - k0, O5], F32, tag=f"w5_{pi}")
                    nc.sync.dma_start(out=t, in_=w_d["w5"][k0:k1, :])
                    w5_sb.append(t)
                sh5 = []
                for ot in range(2):
                    t = sb.tile([128, 1], F32, tag=f"sh5{ot}")
                    nc.sync.dma_start(out=t, in_=w_d["sh5"][ot * 128:(ot + 1) * 128, :])
                    sh5.append(t)
                for ot in range(2):
                    o0 = ot * 128
                    y_ps = ps.tile([128, N], F32, tag="mm")
                    for c in range(4):
                        for pi, (xp, k0, k1) in enumerate(krows):
                            nc.tensor.matmul(
                                y_ps[:, c * 512:(c + 1) * 512],
                                w5_sb[pi][:, o0:o0 + 128],
                                xp[0:k1 - k0, c * 512:(c + 1) * 512],
                                start=(pi == 0), stop=(pi == len(krows) - 1))
                    z5 = sb.tile([128, N], F32, tag="work2048")
                    nc.scalar.activation(out=z5, in_=y_ps, func=AF.Identity,
                                         bias=sh5[ot], scale=1.0)
                    red = sb.tile([128, 1], F32, tag="red5")
                    nc.vector.tensor_reduce(out=red, in_=z5, axis=AX.X, op=OP.max)
                    nc.vector.scalar_tensor_tensor(out=red, in0=red, scalar=0.2,
                                                   in1=red, op0=OP.mult, op1=OP.max)
                    nc.sync.dma_start(
                        out=bass.AP(tensor=out_d.tensor, offset=out_d.offset + s * O5 + o0,
                                    ap=[[1, 128], [1, 1]]),
                        in_=red)
    nc.compile()
    return nc


def fold_weights(inputs):
    """Host-side prep: fold eval-mode BN into the conv weights."""
    folded = {}
    for i in range(1, 6):
        W = np.asarray(inputs[f"W{i}"], np.float32)
        g = np.asarray(inputs[f"g{i}"], np.float32)
        b = np.asarray(inputs[f"b{i}"], np.float32)
        m = np.asarray(inputs[f"m{i}"], np.float32)
        v = np.asarray(inputs[f"v{i}"], np.float32)
        scale = g / np.sqrt(v + EPS)
        shift = b - m * scale
        if i < 5:
            O, C2 = W.shape
            C = C2 // 2
            Wa = W[:, :C]          # acts on (x_j - x_n)
            Wb = W[:, C:]          # acts on x_n
            folded[f"wa{i}"] = np.ascontiguousarray((scale[:, None] * Wa).T)         # [C, O]
            folded[f"wb{i}"] = np.ascontiguousarray((scale[:, None] * (Wb - Wa)).T)  # [C, O]
            folded[f"sh{i}"] = np.ascontiguousarray(shift.reshape(-1, 1))
        else:
            folded["w5"] = np.ascontiguousarray((scale[:, None] * W).T)  # [512, 256]
            folded["sh5"] = np.ascontiguousarray(shift.reshape(-1, 1))
    return folded


_PROGRAM_CACHE = {}


def get_program(num_devices=NCORES, debug_idx=False, repeat=1):
    key = (num_devices, debug_idx, repeat)
    if key not in _PROGRAM_CACHE:
        _PROGRAM_CACHE[key] = build_program(num_devices, debug_idx, repeat)
    return _PROGRAM_CACHE[key]


def make_in_maps(inputs):
    pc = np.asarray(inputs["object_pc"], np.float32)        # [16, 2048, 3]
    xt = np.ascontiguousarray(pc.transpose(0, 2, 1))        # [16, 3, 2048]
    folded = fold_weights(inputs)
    ident = np.eye(128, dtype=np.float32)
    ones = np.ones((1, N), dtype=np.float32)
    in_maps = []
    for c in range(NCORES):
        m = {"x": np.ascontiguousarray(xt[c * SPC:(c + 1) * SPC]),
             "ident": ident, "ones_row": ones}
        m.update(folded)
        in_maps.append(m)
    return in_maps


def run_once(inputs):
    nc = get_program()
    in_maps = make_in_maps(inputs)
    res = bass_utils.run_bass_kernel_spmd(
        nc, in_maps, core_ids=list(range(NCORES)), trace=False)
    out = np.concatenate([r["out"] for r in res.results], axis=0)  # [16, 256]
    return out.astype(np.float32)


def kernel(**inputs):
    return run_once(inputs)


if __name__ == "__main__":
    t0 = time.time()
    nc = build_program()
    print(f"built+compiled in {time.time()-t0:.1f}s")

